# revision 7
# baseline (speedup 1.0000x reference)
"""Trainium2 Bass kernel for nn_Conv_57853209477126.

Computes relu(conv2d(x.reshape(B*S,1,16,8), k3x3, VALID)) as a GEMM:
  out[n, :] = relu(x[n, :] @ W)   with W[128, 84] built from the 3x3 kernel.

Sharding: pure data parallelism over the batch axis across 8 cores.
Each core receives x pre-transposed to [128 pixels, 32768 images] (bf16),
does 256 matmuls with the image tile as the stationary operand (so the
PSUM output lands in natural [images, 84] layout), applies ReLU on the
Scalar/Vector engines, and DMA-stores fp32 results.
"""

import sys

for _p in ("/opt/trn_rl_repo", "/root/.axon_site/_ro/trn_rl_repo"):
    if _p not in sys.path:
        sys.path.append(_p)

import numpy as np
import ml_dtypes

import concourse.bass as bass
import concourse.bacc as bacc
import concourse.tile as tile
from concourse import mybir
from concourse.bass_utils import run_bass_kernel_spmd

# Problem constants (hardcoded per spec).
B, S = 4096, 64
L, W_IMG = 16, 8
K = 3
OL, OW = L - K + 1, W_IMG - K + 1  # 14, 6
PIX = L * W_IMG  # 128
OUT = OL * OW  # 84
N_CORES = 8
N_TOTAL = B * S  # 262144
PER_CORE = N_TOTAL // N_CORES  # 32768

# Device tiling.
G = 4  # matmul tiles per PSUM group (84*4=336 fp32 <= 512/bank)
GROUP = G * 128  # 512 images per PSUM group
N_GROUPS = PER_CORE // GROUP  # 64
LOAD_GROUPS = 8  # groups per input DMA  (8*512 cols * 2B * 128 = 1 MiB)
STORE_GROUPS = 8  # groups per output DMA (8*512 rows * 84 * 4B = 1.31 MiB)

BF16 = mybir.dt.bfloat16
F32 = mybir.dt.float32

_COMPILED = {}


def _build_w128(kernel_np: np.ndarray) -> np.ndarray:
    """[128, 84] matrix: out_img_flat = in_img_flat @ W."""
    w = np.zeros((PIX, OUT), dtype=np.float32)
    for oy in range(OL):
        for ox in range(OW):
            j = oy * OW + ox
            for ky in range(K):
                for kx in range(K):
                    p = (oy + ky) * W_IMG + (ox + kx)
                    w[p, j] += kernel_np[ky, kx]
    return w


PSUM_BUFS = 8
XIN_BUFS = 3
OUT_BUFS = 3


def _build_nc(trace_scopes: bool = False):
    from concourse.tile import add_dep_helper

    nc = bacc.Bacc(
        "TRN2",
        target_bir_lowering=False,
        debug=False,
        num_devices=N_CORES,
    )
    xt_d = nc.dram_tensor("xt", [PIX, PER_CORE], BF16, kind="ExternalInput").ap()
    w_d = nc.dram_tensor("w", [PIX, OUT], BF16, kind="ExternalInput").ap()
    out_d = nc.dram_tensor("out", [PER_CORE, OUT], F32, kind="ExternalOutput").ap()

    with tile.TileContext(nc) as tc:
        with (
            tc.tile_pool(name="wpool", bufs=1) as wpool,
            tc.tile_pool(name="xin", bufs=XIN_BUFS) as xin,
            tc.tile_pool(name="psum", bufs=PSUM_BUFS, space="PSUM") as psum,
            tc.tile_pool(name="outs", bufs=OUT_BUFS) as outs,
        ):
            w_s = wpool.tile([PIX, OUT], BF16)
            nc.sync.dma_start(w_s[:], w_d)

            relu_insts = {}  # global tile idx -> relu instruction
            store_insts = {}  # chunk idx -> store dma instruction

            n_loads = N_GROUPS // LOAD_GROUPS
            for ts in range(n_loads):
                # relu engine for this whole chunk (keeps the store DMA's
                # wait down to a single semaphore)
                eng = nc.scalar if ts % 2 == 0 else nc.vector

                xa = xin.tile([PIX, LOAD_GROUPS * GROUP], BF16, tag="xa")
                nc.sync.dma_start(
                    xa[:], xt_d[:, ts * LOAD_GROUPS * GROUP :][:, : LOAD_GROUPS * GROUP]
                )
                # store tile covering STORE_GROUPS psum groups
                o_s = outs.tile([128, STORE_GROUPS * G * OUT], F32, tag="os")
                for t2 in range(LOAD_GROUPS):
                    t = ts * LOAD_GROUPS + t2
                    po = psum.tile([128, G * OUT], F32, tag="po")
                    for g in range(G):
                        c0 = t2 * GROUP + g * 128
                        nc.tensor.matmul(
                            po[:, g * OUT : (g + 1) * OUT],
                            xa[:, c0 : c0 + 128],
                            w_s[:],
                        )
                    dst = o_s[:, t2 * G * OUT : (t2 + 1) * G * OUT]
                    if ts % 2 == 0:
                        r = nc.scalar.activation(
                            dst, po[:], mybir.ActivationFunctionType.Relu
                        )
                    else:
                        r = nc.vector.tensor_scalar_max(dst, po[:], 0.0)
                    relu_insts[t] = r
                # rows ts*4096 .. (ts+1)*4096; partition n holds rows
                # t2*512 + n*4 + g  (g contiguous) for each t2.
                dst_ap = out_d[ts * STORE_GROUPS * GROUP :][
                    : STORE_GROUPS * GROUP
                ].rearrange("(t2 p g) f -> p t2 g f", p=128, g=G)
                src_ap = o_s[:].rearrange(
                    "p (t2 g f) -> p t2 g f", t2=STORE_GROUPS, g=G
                )
                store_insts[ts] = nc.sync.dma_start(dst_ap, src_ap)

    nc.compile()
    return nc


def _prep_inputs(x: np.ndarray, kernel: np.ndarray):
    """Shard + cast + transpose/permute the inputs for the device layout."""
    w128 = _build_w128(np.asarray(kernel, dtype=np.float32))
    w_bf = w128.astype(ml_dtypes.bfloat16)

    xf = np.asarray(x, dtype=np.float32).reshape(N_TOTAL, PIX)
    in_maps = []
    for c in range(N_CORES):
        xc = xf[c * PER_CORE : (c + 1) * PER_CORE]  # [32768, 128]
        # Column layout: column (t*512 + g*128 + n) = image (t*512 + n*4 + g)
        # so that each store partition n gets G contiguous DRAM rows.
        xr = xc.reshape(N_GROUPS, 128, G, PIX)  # [t, n, g, p]
        xt = xr.transpose(3, 0, 2, 1).reshape(PIX, PER_CORE)  # [p, (t g n)]
        xt_bf = np.ascontiguousarray(xt).astype(ml_dtypes.bfloat16)
        in_maps.append({"xt": xt_bf, "w": w_bf})
    return in_maps


def _install_ntff_hook():
    """The agent image's antenv lacks axon_hooks; bass_utils needs it for
    trace=True. Register a ctypes-based hook module (same logic as
    trn_agent_boot.trn_boot._ntff_profile_via_ctypes)."""
    import types
    import ctypes
    import contextlib

    if "antenv.axon_hooks" in sys.modules:
        return True
    so_path = "/opt/axon/libaxon_pjrt.so"
    try:
        lib = ctypes.CDLL(so_path)
    except OSError:
        return False
    if not hasattr(lib, "axon_start_nrt_profile"):
        return False
    lib.axon_start_nrt_profile.argtypes = [
        ctypes.POINTER(ctypes.c_int64),
        ctypes.c_size_t,
    ]
    lib.axon_start_nrt_profile.restype = ctypes.c_int64
    lib.axon_stop_nrt_profile.argtypes = [ctypes.c_char_p]
    lib.axon_stop_nrt_profile.restype = ctypes.c_int64

    @contextlib.contextmanager
    def _hook(output_dir, device_ids):
        import jax

        jax.devices()
        if device_ids:
            ids = (ctypes.c_int64 * len(device_ids))(*device_ids)
            rc = lib.axon_start_nrt_profile(ids, len(device_ids))
        else:
            rc = lib.axon_start_nrt_profile(None, 0)
        if rc != 0:
            raise RuntimeError(f"axon_start_nrt_profile rc={rc}")
        try:
            yield
        finally:
            n = lib.axon_stop_nrt_profile(str(output_dir).encode())
            print(f"ntff profile: {n} file(s) written to {output_dir}")

    mod = types.ModuleType("antenv.axon_hooks")
    mod._hook = _hook
    mod.get_axon_ntff_profile_hook = lambda: _hook
    mod.set_axon_ntff_profile_hook = lambda h: None
    sys.modules["antenv.axon_hooks"] = mod
    import antenv

    antenv.axon_hooks = mod
    return True


def _run(x, kernel, trace=False):
    key = "nc"
    if key not in _COMPILED:
        _COMPILED[key] = _build_nc()
    nc = _COMPILED[key]
    in_maps = _prep_inputs(x, kernel)
    res = run_bass_kernel_spmd(
        nc, in_maps, core_ids=list(range(N_CORES)), trace=trace
    )
    outs = [res.results[c]["out"] for c in range(N_CORES)]
    full = np.concatenate(outs, axis=0).reshape(B, S, OUT)
    return full.astype(np.float32), res


def kernel(x, kernel):
    out, _ = _run(x, kernel, trace=False)
    return out


def kernel_traced(x, kernel):
    """Same as kernel() but also returns BassKernelResults with trace info."""
    ok = _install_ntff_hook()
    if not ok:
        print("WARNING: could not install NTFF hook; running untraced")
    return _run(x, kernel, trace=ok)


# revision 10
# speedup vs baseline: 1.3104x; 1.3104x over previous
"""Trainium2 Bass kernel for nn_Conv_57853209477126.

Computes relu(conv2d(x.reshape(B*S,1,16,8), k3x3, VALID)) as a GEMM:
  out[n, :] = relu(x[n, :] @ W)   with W[128, 84] built from the 3x3 kernel.

Sharding: pure data parallelism over the batch axis across 8 cores.
Each core receives x pre-transposed to [128 pixels, 32768 images] (bf16),
does 256 matmuls with the image tile as the stationary operand (so the
PSUM output lands in natural [images, 84] layout), applies ReLU on the
Scalar/Vector engines, and DMA-stores fp32 results.
"""

import sys

for _p in ("/opt/trn_rl_repo", "/root/.axon_site/_ro/trn_rl_repo"):
    if _p not in sys.path:
        sys.path.append(_p)

import numpy as np
import ml_dtypes

import concourse.bass as bass
import concourse.bacc as bacc
import concourse.tile as tile
from concourse import mybir
from concourse.bass_utils import run_bass_kernel_spmd

# Problem constants (hardcoded per spec).
B, S = 4096, 64
L, W_IMG = 16, 8
K = 3
OL, OW = L - K + 1, W_IMG - K + 1  # 14, 6
PIX = L * W_IMG  # 128
OUT = OL * OW  # 84
N_CORES = 8
N_TOTAL = B * S  # 262144
PER_CORE = N_TOTAL // N_CORES  # 32768

# Device tiling.
G = 4  # matmul tiles per PSUM group (84*4=336 fp32 <= 512/bank)
GROUP = G * 128  # 512 images per PSUM group
N_GROUPS = PER_CORE // GROUP  # 64
LOAD_GROUPS = 8  # groups per input DMA  (8*512 cols * 2B * 128 = 1 MiB)
STORE_GROUPS = 16  # groups per output DMA (16*512 rows * 84 * 2B = 1.31 MiB bf16)

BF16 = mybir.dt.bfloat16
F32 = mybir.dt.float32
OUT_DT = BF16  # store dtype; host upcasts to fp32
OUT_NP = ml_dtypes.bfloat16

_COMPILED = {}


def _build_w128(kernel_np: np.ndarray) -> np.ndarray:
    """[128, 84] matrix: out_img_flat = in_img_flat @ W."""
    w = np.zeros((PIX, OUT), dtype=np.float32)
    for oy in range(OL):
        for ox in range(OW):
            j = oy * OW + ox
            for ky in range(K):
                for kx in range(K):
                    p = (oy + ky) * W_IMG + (ox + kx)
                    w[p, j] += kernel_np[ky, kx]
    return w


PSUM_BUFS = 8
XIN_BUFS = 3
OUT_BUFS = 3


def _build_nc(trace_scopes: bool = False):
    from concourse.tile import add_dep_helper

    nc = bacc.Bacc(
        "TRN2",
        target_bir_lowering=False,
        debug=False,
        num_devices=N_CORES,
    )
    xt_d = nc.dram_tensor("xt", [PIX, PER_CORE], BF16, kind="ExternalInput").ap()
    w_d = nc.dram_tensor("w", [PIX, OUT], BF16, kind="ExternalInput").ap()
    out_d = nc.dram_tensor("out", [PER_CORE, OUT], OUT_DT, kind="ExternalOutput").ap()

    with tile.TileContext(nc) as tc:
        with (
            tc.tile_pool(name="wpool", bufs=1) as wpool,
            tc.tile_pool(name="xin", bufs=XIN_BUFS) as xin,
            tc.tile_pool(name="psum", bufs=PSUM_BUFS, space="PSUM") as psum,
            tc.tile_pool(name="outs", bufs=OUT_BUFS) as outs,
        ):
            w_s = wpool.tile([PIX, OUT], BF16)
            nc.sync.dma_start(w_s[:], w_d)

            n_stores = N_GROUPS // STORE_GROUPS
            loads_per_store = STORE_GROUPS // LOAD_GROUPS
            for ss in range(n_stores):
                # relu engine alternates per store chunk so each store DMA
                # waits on a single semaphore
                use_act = ss % 2 == 0
                # store tile covering STORE_GROUPS psum groups
                o_s = outs.tile([128, STORE_GROUPS * G * OUT], OUT_DT, tag="os")
                for ls in range(loads_per_store):
                    ts = ss * loads_per_store + ls
                    xa = xin.tile([PIX, LOAD_GROUPS * GROUP], BF16, tag="xa")
                    nc.sync.dma_start(
                        xa[:],
                        xt_d[:, ts * LOAD_GROUPS * GROUP :][:, : LOAD_GROUPS * GROUP],
                    )
                    for t2 in range(LOAD_GROUPS):
                        tg = ls * LOAD_GROUPS + t2  # group idx within store chunk
                        po = psum.tile([128, G * OUT], F32, tag="po")
                        for g in range(G):
                            c0 = t2 * GROUP + g * 128
                            nc.tensor.matmul(
                                po[:, g * OUT : (g + 1) * OUT],
                                xa[:, c0 : c0 + 128],
                                w_s[:],
                            )
                        dst = o_s[:, tg * G * OUT : (tg + 1) * G * OUT]
                        if use_act:
                            nc.scalar.activation(
                                dst, po[:], mybir.ActivationFunctionType.Relu
                            )
                        else:
                            nc.vector.tensor_scalar_max(dst, po[:], 0.0)
                # rows ss*8192 .. (ss+1)*8192; partition n holds rows
                # t2*512 + n*4 + g  (g contiguous) for each of the 16 groups.
                dst_ap = out_d[ss * STORE_GROUPS * GROUP :][
                    : STORE_GROUPS * GROUP
                ].rearrange("(t2 p g) f -> p t2 g f", p=128, g=G)
                src_ap = o_s[:].rearrange(
                    "p (t2 g f) -> p t2 g f", t2=STORE_GROUPS, g=G
                )
                nc.sync.dma_start(dst_ap, src_ap)

    nc.compile()
    return nc


def _prep_inputs(x: np.ndarray, kernel: np.ndarray):
    """Shard + cast + transpose/permute the inputs for the device layout."""
    w128 = _build_w128(np.asarray(kernel, dtype=np.float32))
    w_bf = w128.astype(ml_dtypes.bfloat16)

    xf = np.asarray(x, dtype=np.float32).reshape(N_TOTAL, PIX)
    in_maps = []
    for c in range(N_CORES):
        xc = xf[c * PER_CORE : (c + 1) * PER_CORE]  # [32768, 128]
        # Column layout: column (t*512 + g*128 + n) = image (t*512 + n*4 + g)
        # so that each store partition n gets G contiguous DRAM rows.
        xr = xc.reshape(N_GROUPS, 128, G, PIX)  # [t, n, g, p]
        xt = xr.transpose(3, 0, 2, 1).reshape(PIX, PER_CORE)  # [p, (t g n)]
        xt_bf = np.ascontiguousarray(xt).astype(ml_dtypes.bfloat16)
        in_maps.append({"xt": xt_bf, "w": w_bf})
    return in_maps


def _install_ntff_hook():
    """The agent image's antenv lacks axon_hooks; bass_utils needs it for
    trace=True. Register a ctypes-based hook module (same logic as
    trn_agent_boot.trn_boot._ntff_profile_via_ctypes)."""
    import types
    import ctypes
    import contextlib

    if "antenv.axon_hooks" in sys.modules:
        return True
    so_path = "/opt/axon/libaxon_pjrt.so"
    try:
        lib = ctypes.CDLL(so_path)
    except OSError:
        return False
    if not hasattr(lib, "axon_start_nrt_profile"):
        return False
    lib.axon_start_nrt_profile.argtypes = [
        ctypes.POINTER(ctypes.c_int64),
        ctypes.c_size_t,
    ]
    lib.axon_start_nrt_profile.restype = ctypes.c_int64
    lib.axon_stop_nrt_profile.argtypes = [ctypes.c_char_p]
    lib.axon_stop_nrt_profile.restype = ctypes.c_int64

    @contextlib.contextmanager
    def _hook(output_dir, device_ids):
        import jax

        jax.devices()
        if device_ids:
            ids = (ctypes.c_int64 * len(device_ids))(*device_ids)
            rc = lib.axon_start_nrt_profile(ids, len(device_ids))
        else:
            rc = lib.axon_start_nrt_profile(None, 0)
        if rc != 0:
            raise RuntimeError(f"axon_start_nrt_profile rc={rc}")
        try:
            yield
        finally:
            n = lib.axon_stop_nrt_profile(str(output_dir).encode())
            print(f"ntff profile: {n} file(s) written to {output_dir}")

    mod = types.ModuleType("antenv.axon_hooks")
    mod._hook = _hook
    mod.get_axon_ntff_profile_hook = lambda: _hook
    mod.set_axon_ntff_profile_hook = lambda h: None
    sys.modules["antenv.axon_hooks"] = mod
    import antenv

    antenv.axon_hooks = mod
    return True


def _run(x, kernel, trace=False):
    key = "nc"
    if key not in _COMPILED:
        _COMPILED[key] = _build_nc()
    nc = _COMPILED[key]
    in_maps = _prep_inputs(x, kernel)
    res = run_bass_kernel_spmd(
        nc, in_maps, core_ids=list(range(N_CORES)), trace=trace
    )
    outs = [np.asarray(res.results[c]["out"]) for c in range(N_CORES)]
    full = np.concatenate(outs, axis=0).astype(np.float32).reshape(B, S, OUT)
    return full, res


def kernel(x, kernel):
    out, _ = _run(x, kernel, trace=False)
    return out


def kernel_traced(x, kernel):
    """Same as kernel() but also returns BassKernelResults with trace info."""
    ok = _install_ntff_hook()
    if not ok:
        print("WARNING: could not install NTFF hook; running untraced")
    return _run(x, kernel, trace=ok)


# revision 13
# speedup vs baseline: 1.3759x; 1.0499x over previous
"""Trainium2 Bass kernel for nn_Conv_57853209477126.

Computes relu(conv2d(x.reshape(B*S,1,16,8), k3x3, VALID)) as a GEMM:
  out[n, :] = relu(x[n, :] @ W)   with W[128, 84] built from the 3x3 kernel.

Sharding: pure data parallelism over the batch axis across 8 cores.
Each core receives x pre-transposed to [128 pixels, 32768 images] (bf16),
does 256 matmuls with the image tile as the stationary operand (so the
PSUM output lands in natural [images, 84] layout), applies ReLU on the
Scalar/Vector engines, and DMA-stores fp32 results.
"""

import sys

for _p in ("/opt/trn_rl_repo", "/root/.axon_site/_ro/trn_rl_repo"):
    if _p not in sys.path:
        sys.path.append(_p)

import numpy as np
import ml_dtypes

import concourse.bass as bass
import concourse.bacc as bacc
import concourse.tile as tile
from concourse import mybir
from concourse.bass_utils import run_bass_kernel_spmd

# Problem constants (hardcoded per spec).
B, S = 4096, 64
L, W_IMG = 16, 8
K = 3
OL, OW = L - K + 1, W_IMG - K + 1  # 14, 6
PIX = L * W_IMG  # 128
OUT = OL * OW  # 84
N_CORES = 8
N_TOTAL = B * S  # 262144
PER_CORE = N_TOTAL // N_CORES  # 32768

# Device tiling.
G = 4  # matmul tiles per PSUM group (84*4=336 fp32 <= 512/bank)
GROUP = G * 128  # 512 images per PSUM group
N_GROUPS = PER_CORE // GROUP  # 64
LOAD_GROUPS = 16  # groups per input DMA  (16*512 cols * 2B * 128 = 2 MiB)
STORE_GROUPS = 16  # groups per output DMA (16*512 rows * 84 * 2B = 1.31 MiB bf16)
INTER = 8  # psum groups interleaved per partition row-block in the store
# layout: DRAM row (within a store chunk) = u*4096 + n*32 + h*4 + g
# where t2 = u*8 + h is the group index within the chunk, n the MM lane,
# g the MM index within the psum group. Store descriptor run = 32 rows
# * 168B = 5376B contiguous per (partition, u).

BF16 = mybir.dt.bfloat16
F32 = mybir.dt.float32
OUT_DT = BF16  # store dtype; host upcasts to fp32
OUT_NP = ml_dtypes.bfloat16

_COMPILED = {}


def _build_w128(kernel_np: np.ndarray) -> np.ndarray:
    """[128, 84] matrix: out_img_flat = in_img_flat @ W."""
    w = np.zeros((PIX, OUT), dtype=np.float32)
    for oy in range(OL):
        for ox in range(OW):
            j = oy * OW + ox
            for ky in range(K):
                for kx in range(K):
                    p = (oy + ky) * W_IMG + (ox + kx)
                    w[p, j] += kernel_np[ky, kx]
    return w


PSUM_BUFS = 8
XIN_BUFS = 3
OUT_BUFS = 3


def _build_nc(trace_scopes: bool = False):
    from concourse.tile import add_dep_helper

    nc = bacc.Bacc(
        "TRN2",
        target_bir_lowering=False,
        debug=False,
        num_devices=N_CORES,
    )
    xt_d = nc.dram_tensor("xt", [PIX, PER_CORE], BF16, kind="ExternalInput").ap()
    w_d = nc.dram_tensor("w", [PIX, OUT], BF16, kind="ExternalInput").ap()
    out_d = nc.dram_tensor("out", [PER_CORE, OUT], OUT_DT, kind="ExternalOutput").ap()

    with tile.TileContext(nc) as tc:
        with (
            tc.tile_pool(name="wpool", bufs=1) as wpool,
            tc.tile_pool(name="xin", bufs=XIN_BUFS) as xin,
            tc.tile_pool(name="psum", bufs=PSUM_BUFS, space="PSUM") as psum,
            tc.tile_pool(name="outs", bufs=OUT_BUFS) as outs,
        ):
            w_s = wpool.tile([PIX, OUT], BF16)
            nc.sync.dma_start(w_s[:], w_d)

            n_stores = N_GROUPS // STORE_GROUPS
            loads_per_store = STORE_GROUPS // LOAD_GROUPS
            for ss in range(n_stores):
                # relu engine alternates per store chunk so each store DMA
                # waits on a single semaphore
                use_act = ss % 2 == 0
                # store tile covering STORE_GROUPS psum groups
                o_s = outs.tile([128, STORE_GROUPS * G * OUT], OUT_DT, tag="os")
                for ls in range(loads_per_store):
                    ts = ss * loads_per_store + ls
                    xa = xin.tile([PIX, LOAD_GROUPS * GROUP], BF16, tag="xa")
                    nc.sync.dma_start(
                        xa[:],
                        xt_d[:, ts * LOAD_GROUPS * GROUP :][:, : LOAD_GROUPS * GROUP],
                    )
                    for t2 in range(LOAD_GROUPS):
                        tg = ls * LOAD_GROUPS + t2  # group idx within store chunk
                        po = psum.tile([128, G * OUT], F32, tag="po")
                        for g in range(G):
                            c0 = t2 * GROUP + g * 128
                            nc.tensor.matmul(
                                po[:, g * OUT : (g + 1) * OUT],
                                xa[:, c0 : c0 + 128],
                                w_s[:],
                            )
                        dst = o_s[:, tg * G * OUT : (tg + 1) * G * OUT]
                        if use_act:
                            nc.scalar.activation(
                                dst, po[:], mybir.ActivationFunctionType.Relu
                            )
                        else:
                            nc.vector.tensor_scalar_max(dst, po[:], 0.0)
                # rows ss*8192 .. (ss+1)*8192; partition n holds rows
                # u*4096 + n*32 + h*4 + g -> 32 consecutive rows per (n, u),
                # i.e. one 5376B contiguous store run per descriptor.
                n_u = STORE_GROUPS // INTER
                dst_ap = out_d[ss * STORE_GROUPS * GROUP :][
                    : STORE_GROUPS * GROUP
                ].rearrange("(u p h g) f -> p u (h g f)", p=128, h=INTER, g=G)
                src_ap = o_s[:].rearrange("p (u r) -> p u r", u=n_u)
                nc.sync.dma_start(dst_ap, src_ap)

    nc.compile()
    return nc


def _prep_inputs(x: np.ndarray, kernel: np.ndarray):
    """Shard + cast + transpose/permute the inputs for the device layout."""
    w128 = _build_w128(np.asarray(kernel, dtype=np.float32))
    w_bf = w128.astype(ml_dtypes.bfloat16)

    xf = np.asarray(x, dtype=np.float32).reshape(N_TOTAL, PIX)
    # Column layout: xt column c = t*512 + g*128 + n holds the image that the
    # store writes to DRAM row ss*8192 + u*4096 + n*32 + h*4 + g, where
    # t = ss*16 + u*8 + h. Build the permutation via a reshape/transpose:
    # images viewed as [ss, u, n, h, g, p] -> column order (ss, u, h, g, n).
    n_ss = N_GROUPS // STORE_GROUPS
    n_u = STORE_GROUPS // INTER
    in_maps = []
    for c in range(N_CORES):
        xc = xf[c * PER_CORE : (c + 1) * PER_CORE]  # [32768, 128]
        xr = xc.reshape(n_ss, n_u, 128, INTER, G, PIX)  # [ss, u, n, h, g, p]
        # -> [p, ss, u, h, g, n]
        xt = xr.transpose(5, 0, 1, 3, 4, 2).reshape(PIX, PER_CORE)
        xt_bf = np.ascontiguousarray(xt).astype(ml_dtypes.bfloat16)
        in_maps.append({"xt": xt_bf, "w": w_bf})
    return in_maps


def _install_ntff_hook():
    """The agent image's antenv lacks axon_hooks; bass_utils needs it for
    trace=True. Register a ctypes-based hook module (same logic as
    trn_agent_boot.trn_boot._ntff_profile_via_ctypes)."""
    import types
    import ctypes
    import contextlib

    if "antenv.axon_hooks" in sys.modules:
        return True
    so_path = "/opt/axon/libaxon_pjrt.so"
    try:
        lib = ctypes.CDLL(so_path)
    except OSError:
        return False
    if not hasattr(lib, "axon_start_nrt_profile"):
        return False
    lib.axon_start_nrt_profile.argtypes = [
        ctypes.POINTER(ctypes.c_int64),
        ctypes.c_size_t,
    ]
    lib.axon_start_nrt_profile.restype = ctypes.c_int64
    lib.axon_stop_nrt_profile.argtypes = [ctypes.c_char_p]
    lib.axon_stop_nrt_profile.restype = ctypes.c_int64

    @contextlib.contextmanager
    def _hook(output_dir, device_ids):
        import jax

        jax.devices()
        if device_ids:
            ids = (ctypes.c_int64 * len(device_ids))(*device_ids)
            rc = lib.axon_start_nrt_profile(ids, len(device_ids))
        else:
            rc = lib.axon_start_nrt_profile(None, 0)
        if rc != 0:
            raise RuntimeError(f"axon_start_nrt_profile rc={rc}")
        try:
            yield
        finally:
            n = lib.axon_stop_nrt_profile(str(output_dir).encode())
            print(f"ntff profile: {n} file(s) written to {output_dir}")

    mod = types.ModuleType("antenv.axon_hooks")
    mod._hook = _hook
    mod.get_axon_ntff_profile_hook = lambda: _hook
    mod.set_axon_ntff_profile_hook = lambda h: None
    sys.modules["antenv.axon_hooks"] = mod
    import antenv

    antenv.axon_hooks = mod
    return True


def _run(x, kernel, trace=False):
    key = "nc"
    if key not in _COMPILED:
        _COMPILED[key] = _build_nc()
    nc = _COMPILED[key]
    in_maps = _prep_inputs(x, kernel)
    res = run_bass_kernel_spmd(
        nc, in_maps, core_ids=list(range(N_CORES)), trace=trace
    )
    outs = [np.asarray(res.results[c]["out"]) for c in range(N_CORES)]
    full = np.concatenate(outs, axis=0).astype(np.float32).reshape(B, S, OUT)
    return full, res


def kernel(x, kernel):
    out, _ = _run(x, kernel, trace=False)
    return out


def kernel_traced(x, kernel):
    """Same as kernel() but also returns BassKernelResults with trace info."""
    ok = _install_ntff_hook()
    if not ok:
        print("WARNING: could not install NTFF hook; running untraced")
    return _run(x, kernel, trace=ok)


# revision 14
# speedup vs baseline: 1.5037x; 1.0929x over previous
"""Trainium2 Bass kernel for nn_Conv_57853209477126.

Computes relu(conv2d(x.reshape(B*S,1,16,8), k3x3, VALID)) as a GEMM:
  out[n, :] = relu(x[n, :] @ W)   with W[128, 84] built from the 3x3 kernel.

Sharding: pure data parallelism over the batch axis across 8 cores.
Each core receives x pre-transposed to [128 pixels, 32768 images] (bf16),
does 256 matmuls with the image tile as the stationary operand (so the
PSUM output lands in natural [images, 84] layout), applies ReLU on the
Scalar/Vector engines, and DMA-stores fp32 results.
"""

import sys

for _p in ("/opt/trn_rl_repo", "/root/.axon_site/_ro/trn_rl_repo"):
    if _p not in sys.path:
        sys.path.append(_p)

import numpy as np
import ml_dtypes

import concourse.bass as bass
import concourse.bacc as bacc
import concourse.tile as tile
from concourse import mybir
from concourse.bass_utils import run_bass_kernel_spmd

# Problem constants (hardcoded per spec).
B, S = 4096, 64
L, W_IMG = 16, 8
K = 3
OL, OW = L - K + 1, W_IMG - K + 1  # 14, 6
PIX = L * W_IMG  # 128
OUT = OL * OW  # 84
N_CORES = 8
N_TOTAL = B * S  # 262144
PER_CORE = N_TOTAL // N_CORES  # 32768

# Device tiling.
G = 4  # matmul tiles per PSUM group (84*4=336 fp32 <= 512/bank)
GROUP = G * 128  # 512 images per PSUM group
N_GROUPS = PER_CORE // GROUP  # 64
LOAD_GROUPS = 8  # groups per input DMA  (8*512 cols * 2B * 128 = 1 MiB)
STORE_GROUPS = 16  # groups per output DMA (16*512 rows * 84 * 2B = 1.31 MiB bf16)
INTER = 8  # psum groups interleaved per partition row-block in the store
# layout: DRAM row (within a store chunk) = u*4096 + n*32 + h*4 + g
# where t2 = u*8 + h is the group index within the chunk, n the MM lane,
# g the MM index within the psum group. Store descriptor run = 32 rows
# * 168B = 5376B contiguous per (partition, u).

BF16 = mybir.dt.bfloat16
F32 = mybir.dt.float32
OUT_DT = BF16  # store dtype; host upcasts to fp32
OUT_NP = ml_dtypes.bfloat16

_COMPILED = {}


def _build_w128(kernel_np: np.ndarray) -> np.ndarray:
    """[128, 84] matrix: out_img_flat = in_img_flat @ W."""
    w = np.zeros((PIX, OUT), dtype=np.float32)
    for oy in range(OL):
        for ox in range(OW):
            j = oy * OW + ox
            for ky in range(K):
                for kx in range(K):
                    p = (oy + ky) * W_IMG + (ox + kx)
                    w[p, j] += kernel_np[ky, kx]
    return w


PSUM_BUFS = 8
XIN_BUFS = 3
OUT_BUFS = 3


def _build_nc(trace_scopes: bool = False):
    from concourse.tile import add_dep_helper

    nc = bacc.Bacc(
        "TRN2",
        target_bir_lowering=False,
        debug=False,
        num_devices=N_CORES,
    )
    xt_d = nc.dram_tensor("xt", [PIX, PER_CORE], BF16, kind="ExternalInput").ap()
    w_d = nc.dram_tensor("w", [PIX, OUT], BF16, kind="ExternalInput").ap()
    out_d = nc.dram_tensor("out", [PER_CORE, OUT], OUT_DT, kind="ExternalOutput").ap()

    with tile.TileContext(nc) as tc:
        with (
            tc.tile_pool(name="wpool", bufs=1) as wpool,
            tc.tile_pool(name="xin", bufs=XIN_BUFS) as xin,
            tc.tile_pool(name="psum", bufs=PSUM_BUFS, space="PSUM") as psum,
            tc.tile_pool(name="outs", bufs=OUT_BUFS) as outs,
        ):
            w_s = wpool.tile([PIX, OUT], BF16)
            nc.sync.dma_start(w_s[:], w_d)

            n_stores = N_GROUPS // STORE_GROUPS
            loads_per_store = STORE_GROUPS // LOAD_GROUPS
            for ss in range(n_stores):
                # relu engine alternates per store chunk so each store DMA
                # waits on a single semaphore
                use_act = ss % 2 == 0
                # store tile covering STORE_GROUPS psum groups
                o_s = outs.tile([128, STORE_GROUPS * G * OUT], OUT_DT, tag="os")
                for ls in range(loads_per_store):
                    ts = ss * loads_per_store + ls
                    xa = xin.tile([PIX, LOAD_GROUPS * GROUP], BF16, tag="xa")
                    nc.sync.dma_start(
                        xa[:],
                        xt_d[:, ts * LOAD_GROUPS * GROUP :][:, : LOAD_GROUPS * GROUP],
                    )
                    for t2 in range(LOAD_GROUPS):
                        tg = ls * LOAD_GROUPS + t2  # group idx within store chunk
                        po = psum.tile([128, G * OUT], F32, tag="po")
                        for g in range(G):
                            c0 = t2 * GROUP + g * 128
                            nc.tensor.matmul(
                                po[:, g * OUT : (g + 1) * OUT],
                                xa[:, c0 : c0 + 128],
                                w_s[:],
                            )
                        dst = o_s[:, tg * G * OUT : (tg + 1) * G * OUT]
                        if use_act:
                            nc.scalar.activation(
                                dst, po[:], mybir.ActivationFunctionType.Relu
                            )
                        else:
                            nc.vector.tensor_scalar_max(dst, po[:], 0.0)
                # rows ss*8192 .. (ss+1)*8192; partition n holds rows
                # u*4096 + n*32 + h*4 + g -> 32 consecutive rows per (n, u),
                # i.e. one 5376B contiguous store run per descriptor.
                n_u = STORE_GROUPS // INTER
                dst_ap = out_d[ss * STORE_GROUPS * GROUP :][
                    : STORE_GROUPS * GROUP
                ].rearrange("(u p h g) f -> p u (h g f)", p=128, h=INTER, g=G)
                src_ap = o_s[:].rearrange("p (u r) -> p u r", u=n_u)
                # stores go on the ACT HWDGE ring so a store trigger waiting
                # on a relu semaphore never head-of-line-blocks a load on SP
                nc.scalar.dma_start(dst_ap, src_ap)

    nc.compile()
    return nc


def _prep_inputs(x: np.ndarray, kernel: np.ndarray):
    """Shard + cast + transpose/permute the inputs for the device layout."""
    w128 = _build_w128(np.asarray(kernel, dtype=np.float32))
    w_bf = w128.astype(ml_dtypes.bfloat16)

    xf = np.asarray(x, dtype=np.float32).reshape(N_TOTAL, PIX)
    # Column layout: xt column c = t*512 + g*128 + n holds the image that the
    # store writes to DRAM row ss*8192 + u*4096 + n*32 + h*4 + g, where
    # t = ss*16 + u*8 + h. Build the permutation via a reshape/transpose:
    # images viewed as [ss, u, n, h, g, p] -> column order (ss, u, h, g, n).
    n_ss = N_GROUPS // STORE_GROUPS
    n_u = STORE_GROUPS // INTER
    in_maps = []
    for c in range(N_CORES):
        xc = xf[c * PER_CORE : (c + 1) * PER_CORE]  # [32768, 128]
        xr = xc.reshape(n_ss, n_u, 128, INTER, G, PIX)  # [ss, u, n, h, g, p]
        # -> [p, ss, u, h, g, n]
        xt = xr.transpose(5, 0, 1, 3, 4, 2).reshape(PIX, PER_CORE)
        xt_bf = np.ascontiguousarray(xt).astype(ml_dtypes.bfloat16)
        in_maps.append({"xt": xt_bf, "w": w_bf})
    return in_maps


def _install_ntff_hook():
    """The agent image's antenv lacks axon_hooks; bass_utils needs it for
    trace=True. Register a ctypes-based hook module (same logic as
    trn_agent_boot.trn_boot._ntff_profile_via_ctypes)."""
    import types
    import ctypes
    import contextlib

    if "antenv.axon_hooks" in sys.modules:
        return True
    so_path = "/opt/axon/libaxon_pjrt.so"
    try:
        lib = ctypes.CDLL(so_path)
    except OSError:
        return False
    if not hasattr(lib, "axon_start_nrt_profile"):
        return False
    lib.axon_start_nrt_profile.argtypes = [
        ctypes.POINTER(ctypes.c_int64),
        ctypes.c_size_t,
    ]
    lib.axon_start_nrt_profile.restype = ctypes.c_int64
    lib.axon_stop_nrt_profile.argtypes = [ctypes.c_char_p]
    lib.axon_stop_nrt_profile.restype = ctypes.c_int64

    @contextlib.contextmanager
    def _hook(output_dir, device_ids):
        import jax

        jax.devices()
        if device_ids:
            ids = (ctypes.c_int64 * len(device_ids))(*device_ids)
            rc = lib.axon_start_nrt_profile(ids, len(device_ids))
        else:
            rc = lib.axon_start_nrt_profile(None, 0)
        if rc != 0:
            raise RuntimeError(f"axon_start_nrt_profile rc={rc}")
        try:
            yield
        finally:
            n = lib.axon_stop_nrt_profile(str(output_dir).encode())
            print(f"ntff profile: {n} file(s) written to {output_dir}")

    mod = types.ModuleType("antenv.axon_hooks")
    mod._hook = _hook
    mod.get_axon_ntff_profile_hook = lambda: _hook
    mod.set_axon_ntff_profile_hook = lambda h: None
    sys.modules["antenv.axon_hooks"] = mod
    import antenv

    antenv.axon_hooks = mod
    return True


def _run(x, kernel, trace=False):
    key = "nc"
    if key not in _COMPILED:
        _COMPILED[key] = _build_nc()
    nc = _COMPILED[key]
    in_maps = _prep_inputs(x, kernel)
    res = run_bass_kernel_spmd(
        nc, in_maps, core_ids=list(range(N_CORES)), trace=trace
    )
    outs = [np.asarray(res.results[c]["out"]) for c in range(N_CORES)]
    full = np.concatenate(outs, axis=0).astype(np.float32).reshape(B, S, OUT)
    return full, res


def kernel(x, kernel):
    out, _ = _run(x, kernel, trace=False)
    return out


def kernel_traced(x, kernel):
    """Same as kernel() but also returns BassKernelResults with trace info."""
    ok = _install_ntff_hook()
    if not ok:
        print("WARNING: could not install NTFF hook; running untraced")
    return _run(x, kernel, trace=ok)


# revision 15
# speedup vs baseline: 1.5647x; 1.0406x over previous
"""Trainium2 Bass kernel for nn_Conv_57853209477126.

Computes relu(conv2d(x.reshape(B*S,1,16,8), k3x3, VALID)) as a GEMM:
  out[n, :] = relu(x[n, :] @ W)   with W[128, 84] built from the 3x3 kernel.

Sharding: pure data parallelism over the batch axis across 8 cores.
Each core receives x pre-transposed to [128 pixels, 32768 images] (bf16),
does 256 matmuls with the image tile as the stationary operand (so the
PSUM output lands in natural [images, 84] layout), applies ReLU on the
Scalar/Vector engines, and DMA-stores fp32 results.
"""

import sys

for _p in ("/opt/trn_rl_repo", "/root/.axon_site/_ro/trn_rl_repo"):
    if _p not in sys.path:
        sys.path.append(_p)

import numpy as np
import ml_dtypes

import concourse.bass as bass
import concourse.bacc as bacc
import concourse.tile as tile
from concourse import mybir
from concourse.bass_utils import run_bass_kernel_spmd

# Problem constants (hardcoded per spec).
B, S = 4096, 64
L, W_IMG = 16, 8
K = 3
OL, OW = L - K + 1, W_IMG - K + 1  # 14, 6
PIX = L * W_IMG  # 128
OUT = OL * OW  # 84
N_CORES = 8
N_TOTAL = B * S  # 262144
PER_CORE = N_TOTAL // N_CORES  # 32768

# Device tiling.
G = 4  # matmul tiles per PSUM group (84*4=336 fp32 <= 512/bank)
GROUP = G * 128  # 512 images per PSUM group
N_GROUPS = PER_CORE // GROUP  # 64
LOAD_GROUPS = 8  # groups per input DMA  (8*512 cols * 2B * 128 = 1 MiB)
STORE_GROUPS = 8  # groups per output DMA (8*512 rows * 84 * 2B = 688 KiB bf16)
INTER = 8  # psum groups interleaved per partition row-block in the store
# layout: DRAM row (within a store chunk) = u*4096 + n*32 + h*4 + g
# where t2 = u*8 + h is the group index within the chunk, n the MM lane,
# g the MM index within the psum group. Store descriptor run = 32 rows
# * 168B = 5376B contiguous per (partition, u).

BF16 = mybir.dt.bfloat16
F32 = mybir.dt.float32
OUT_DT = BF16  # store dtype; host upcasts to fp32
OUT_NP = ml_dtypes.bfloat16

_COMPILED = {}


def _build_w128(kernel_np: np.ndarray) -> np.ndarray:
    """[128, 84] matrix: out_img_flat = in_img_flat @ W."""
    w = np.zeros((PIX, OUT), dtype=np.float32)
    for oy in range(OL):
        for ox in range(OW):
            j = oy * OW + ox
            for ky in range(K):
                for kx in range(K):
                    p = (oy + ky) * W_IMG + (ox + kx)
                    w[p, j] += kernel_np[ky, kx]
    return w


PSUM_BUFS = 8
XIN_BUFS = 4
OUT_BUFS = 4


def _build_nc(trace_scopes: bool = False):
    from concourse.tile import add_dep_helper

    nc = bacc.Bacc(
        "TRN2",
        target_bir_lowering=False,
        debug=False,
        num_devices=N_CORES,
    )
    xt_d = nc.dram_tensor("xt", [PIX, PER_CORE], BF16, kind="ExternalInput").ap()
    w_d = nc.dram_tensor("w", [PIX, OUT], BF16, kind="ExternalInput").ap()
    out_d = nc.dram_tensor("out", [PER_CORE, OUT], OUT_DT, kind="ExternalOutput").ap()

    with tile.TileContext(nc) as tc:
        with (
            tc.tile_pool(name="wpool", bufs=1) as wpool,
            tc.tile_pool(name="xin", bufs=XIN_BUFS) as xin,
            tc.tile_pool(name="psum", bufs=PSUM_BUFS, space="PSUM") as psum,
            tc.tile_pool(name="outs", bufs=OUT_BUFS) as outs,
        ):
            w_s = wpool.tile([PIX, OUT], BF16)
            nc.sync.dma_start(w_s[:], w_d)

            n_stores = N_GROUPS // STORE_GROUPS
            loads_per_store = STORE_GROUPS // LOAD_GROUPS
            for ss in range(n_stores):
                # relu engine alternates per store chunk so each store DMA
                # waits on a single semaphore
                use_act = ss % 2 == 0
                # store tile covering STORE_GROUPS psum groups
                o_s = outs.tile([128, STORE_GROUPS * G * OUT], OUT_DT, tag="os")
                for ls in range(loads_per_store):
                    ts = ss * loads_per_store + ls
                    xa = xin.tile([PIX, LOAD_GROUPS * GROUP], BF16, tag="xa")
                    nc.sync.dma_start(
                        xa[:],
                        xt_d[:, ts * LOAD_GROUPS * GROUP :][:, : LOAD_GROUPS * GROUP],
                    )
                    for t2 in range(LOAD_GROUPS):
                        tg = ls * LOAD_GROUPS + t2  # group idx within store chunk
                        po = psum.tile([128, G * OUT], F32, tag="po")
                        for g in range(G):
                            c0 = t2 * GROUP + g * 128
                            nc.tensor.matmul(
                                po[:, g * OUT : (g + 1) * OUT],
                                xa[:, c0 : c0 + 128],
                                w_s[:],
                            )
                        dst = o_s[:, tg * G * OUT : (tg + 1) * G * OUT]
                        if use_act:
                            nc.scalar.activation(
                                dst, po[:], mybir.ActivationFunctionType.Relu
                            )
                        else:
                            nc.vector.tensor_scalar_max(dst, po[:], 0.0)
                # rows ss*8192 .. (ss+1)*8192; partition n holds rows
                # u*4096 + n*32 + h*4 + g -> 32 consecutive rows per (n, u),
                # i.e. one 5376B contiguous store run per descriptor.
                n_u = STORE_GROUPS // INTER
                dst_ap = out_d[ss * STORE_GROUPS * GROUP :][
                    : STORE_GROUPS * GROUP
                ].rearrange("(u p h g) f -> p u (h g f)", p=128, h=INTER, g=G)
                src_ap = o_s[:].rearrange("p (u r) -> p u r", u=n_u)
                # stores go on the ACT HWDGE ring so a store trigger waiting
                # on a relu semaphore never head-of-line-blocks a load on SP
                nc.scalar.dma_start(dst_ap, src_ap)

    nc.compile()
    return nc


def _prep_inputs(x: np.ndarray, kernel: np.ndarray):
    """Shard + cast + transpose/permute the inputs for the device layout."""
    w128 = _build_w128(np.asarray(kernel, dtype=np.float32))
    w_bf = w128.astype(ml_dtypes.bfloat16)

    xf = np.asarray(x, dtype=np.float32).reshape(N_TOTAL, PIX)
    # Column layout: xt column c = t*512 + g*128 + n holds the image that the
    # store writes to DRAM row ss*8192 + u*4096 + n*32 + h*4 + g, where
    # t = ss*16 + u*8 + h. Build the permutation via a reshape/transpose:
    # images viewed as [ss, u, n, h, g, p] -> column order (ss, u, h, g, n).
    n_ss = N_GROUPS // STORE_GROUPS
    n_u = STORE_GROUPS // INTER
    in_maps = []
    for c in range(N_CORES):
        xc = xf[c * PER_CORE : (c + 1) * PER_CORE]  # [32768, 128]
        xr = xc.reshape(n_ss, n_u, 128, INTER, G, PIX)  # [ss, u, n, h, g, p]
        # -> [p, ss, u, h, g, n]
        xt = xr.transpose(5, 0, 1, 3, 4, 2).reshape(PIX, PER_CORE)
        xt_bf = np.ascontiguousarray(xt).astype(ml_dtypes.bfloat16)
        in_maps.append({"xt": xt_bf, "w": w_bf})
    return in_maps


def _install_ntff_hook():
    """The agent image's antenv lacks axon_hooks; bass_utils needs it for
    trace=True. Register a ctypes-based hook module (same logic as
    trn_agent_boot.trn_boot._ntff_profile_via_ctypes)."""
    import types
    import ctypes
    import contextlib

    if "antenv.axon_hooks" in sys.modules:
        return True
    so_path = "/opt/axon/libaxon_pjrt.so"
    try:
        lib = ctypes.CDLL(so_path)
    except OSError:
        return False
    if not hasattr(lib, "axon_start_nrt_profile"):
        return False
    lib.axon_start_nrt_profile.argtypes = [
        ctypes.POINTER(ctypes.c_int64),
        ctypes.c_size_t,
    ]
    lib.axon_start_nrt_profile.restype = ctypes.c_int64
    lib.axon_stop_nrt_profile.argtypes = [ctypes.c_char_p]
    lib.axon_stop_nrt_profile.restype = ctypes.c_int64

    @contextlib.contextmanager
    def _hook(output_dir, device_ids):
        import jax

        jax.devices()
        if device_ids:
            ids = (ctypes.c_int64 * len(device_ids))(*device_ids)
            rc = lib.axon_start_nrt_profile(ids, len(device_ids))
        else:
            rc = lib.axon_start_nrt_profile(None, 0)
        if rc != 0:
            raise RuntimeError(f"axon_start_nrt_profile rc={rc}")
        try:
            yield
        finally:
            n = lib.axon_stop_nrt_profile(str(output_dir).encode())
            print(f"ntff profile: {n} file(s) written to {output_dir}")

    mod = types.ModuleType("antenv.axon_hooks")
    mod._hook = _hook
    mod.get_axon_ntff_profile_hook = lambda: _hook
    mod.set_axon_ntff_profile_hook = lambda h: None
    sys.modules["antenv.axon_hooks"] = mod
    import antenv

    antenv.axon_hooks = mod
    return True


def _run(x, kernel, trace=False):
    key = "nc"
    if key not in _COMPILED:
        _COMPILED[key] = _build_nc()
    nc = _COMPILED[key]
    in_maps = _prep_inputs(x, kernel)
    res = run_bass_kernel_spmd(
        nc, in_maps, core_ids=list(range(N_CORES)), trace=trace
    )
    outs = [np.asarray(res.results[c]["out"]) for c in range(N_CORES)]
    full = np.concatenate(outs, axis=0).astype(np.float32).reshape(B, S, OUT)
    return full, res


def kernel(x, kernel):
    out, _ = _run(x, kernel, trace=False)
    return out


def kernel_traced(x, kernel):
    """Same as kernel() but also returns BassKernelResults with trace info."""
    ok = _install_ntff_hook()
    if not ok:
        print("WARNING: could not install NTFF hook; running untraced")
    return _run(x, kernel, trace=ok)


# revision 16
# speedup vs baseline: 1.7092x; 1.0924x over previous
"""Trainium2 Bass kernel for nn_Conv_57853209477126.

Computes relu(conv2d(x.reshape(B*S,1,16,8), k3x3, VALID)) as a GEMM:
  out[n, :] = relu(x[n, :] @ W)   with W[128, 84] built from the 3x3 kernel.

Sharding: pure data parallelism over the batch axis across 8 cores.
Each core receives x pre-transposed to [128 pixels, 32768 images] (bf16),
does 256 matmuls with the image tile as the stationary operand (so the
PSUM output lands in natural [images, 84] layout), applies ReLU on the
Scalar/Vector engines, and DMA-stores fp32 results.
"""

import sys

for _p in ("/opt/trn_rl_repo", "/root/.axon_site/_ro/trn_rl_repo"):
    if _p not in sys.path:
        sys.path.append(_p)

import numpy as np
import ml_dtypes

import concourse.bass as bass
import concourse.bacc as bacc
import concourse.tile as tile
from concourse import mybir
from concourse.bass_utils import run_bass_kernel_spmd

# Problem constants (hardcoded per spec).
B, S = 4096, 64
L, W_IMG = 16, 8
K = 3
OL, OW = L - K + 1, W_IMG - K + 1  # 14, 6
PIX = L * W_IMG  # 128
OUT = OL * OW  # 84
N_CORES = 8
N_TOTAL = B * S  # 262144
PER_CORE = N_TOTAL // N_CORES  # 32768

# Device tiling.
G = 4  # matmul tiles per PSUM group (84*4=336 fp32 <= 512/bank)
GROUP = G * 128  # 512 images per PSUM group
N_GROUPS = PER_CORE // GROUP  # 64
LOAD_GROUPS = 8  # groups per input DMA  (8*512 cols * 2B * 128 = 1 MiB)
STORE_GROUPS = 8  # groups per output DMA (8*512 rows * 84 * 2B = 688 KiB bf16)
RELU_SPLIT = True  # alternate relu engine per psum group (not per chunk)
INTER = 8  # psum groups interleaved per partition row-block in the store
# layout: DRAM row (within a store chunk) = u*4096 + n*32 + h*4 + g
# where t2 = u*8 + h is the group index within the chunk, n the MM lane,
# g the MM index within the psum group. Store descriptor run = 32 rows
# * 168B = 5376B contiguous per (partition, u).

BF16 = mybir.dt.bfloat16
F32 = mybir.dt.float32
OUT_DT = BF16  # store dtype; host upcasts to fp32
OUT_NP = ml_dtypes.bfloat16

_COMPILED = {}


def _build_w128(kernel_np: np.ndarray) -> np.ndarray:
    """[128, 84] matrix: out_img_flat = in_img_flat @ W."""
    w = np.zeros((PIX, OUT), dtype=np.float32)
    for oy in range(OL):
        for ox in range(OW):
            j = oy * OW + ox
            for ky in range(K):
                for kx in range(K):
                    p = (oy + ky) * W_IMG + (ox + kx)
                    w[p, j] += kernel_np[ky, kx]
    return w


PSUM_BUFS = 8
XIN_BUFS = 4
OUT_BUFS = 4


def _build_nc(trace_scopes: bool = False):
    from concourse.tile import add_dep_helper

    nc = bacc.Bacc(
        "TRN2",
        target_bir_lowering=False,
        debug=False,
        num_devices=N_CORES,
    )
    xt_d = nc.dram_tensor("xt", [PIX, PER_CORE], BF16, kind="ExternalInput").ap()
    w_d = nc.dram_tensor("w", [PIX, OUT], BF16, kind="ExternalInput").ap()
    out_d = nc.dram_tensor("out", [PER_CORE, OUT], OUT_DT, kind="ExternalOutput").ap()

    with tile.TileContext(nc) as tc:
        with (
            tc.tile_pool(name="wpool", bufs=1) as wpool,
            tc.tile_pool(name="xin", bufs=XIN_BUFS) as xin,
            tc.tile_pool(name="psum", bufs=PSUM_BUFS, space="PSUM") as psum,
            tc.tile_pool(name="outs", bufs=OUT_BUFS) as outs,
        ):
            w_s = wpool.tile([PIX, OUT], BF16)
            nc.sync.dma_start(w_s[:], w_d)

            n_stores = N_GROUPS // STORE_GROUPS
            loads_per_store = STORE_GROUPS // LOAD_GROUPS
            for ss in range(n_stores):
                # relu engine: split per psum group across both engines
                # (bacc legalizes the store DMA's two-semaphore wait)
                # store tile covering STORE_GROUPS psum groups
                o_s = outs.tile([128, STORE_GROUPS * G * OUT], OUT_DT, tag="os")
                for ls in range(loads_per_store):
                    ts = ss * loads_per_store + ls
                    xa = xin.tile([PIX, LOAD_GROUPS * GROUP], BF16, tag="xa")
                    nc.sync.dma_start(
                        xa[:],
                        xt_d[:, ts * LOAD_GROUPS * GROUP :][:, : LOAD_GROUPS * GROUP],
                    )
                    for t2 in range(LOAD_GROUPS):
                        tg = ls * LOAD_GROUPS + t2  # group idx within store chunk
                        po = psum.tile([128, G * OUT], F32, tag="po")
                        for g in range(G):
                            c0 = t2 * GROUP + g * 128
                            nc.tensor.matmul(
                                po[:, g * OUT : (g + 1) * OUT],
                                xa[:, c0 : c0 + 128],
                                w_s[:],
                            )
                        dst = o_s[:, tg * G * OUT : (tg + 1) * G * OUT]
                        if (ss * STORE_GROUPS + tg) % 2 == 0:
                            nc.scalar.activation(
                                dst, po[:], mybir.ActivationFunctionType.Relu
                            )
                        else:
                            nc.vector.tensor_scalar_max(dst, po[:], 0.0)
                # rows ss*8192 .. (ss+1)*8192; partition n holds rows
                # u*4096 + n*32 + h*4 + g -> 32 consecutive rows per (n, u),
                # i.e. one 5376B contiguous store run per descriptor.
                n_u = STORE_GROUPS // INTER
                dst_ap = out_d[ss * STORE_GROUPS * GROUP :][
                    : STORE_GROUPS * GROUP
                ].rearrange("(u p h g) f -> p u (h g f)", p=128, h=INTER, g=G)
                src_ap = o_s[:].rearrange("p (u r) -> p u r", u=n_u)
                # stores go on the ACT HWDGE ring so a store trigger waiting
                # on a relu semaphore never head-of-line-blocks a load on SP
                nc.scalar.dma_start(dst_ap, src_ap)

    nc.compile()
    return nc


def _prep_inputs(x: np.ndarray, kernel: np.ndarray):
    """Shard + cast + transpose/permute the inputs for the device layout."""
    w128 = _build_w128(np.asarray(kernel, dtype=np.float32))
    w_bf = w128.astype(ml_dtypes.bfloat16)

    xf = np.asarray(x, dtype=np.float32).reshape(N_TOTAL, PIX)
    # Column layout: xt column c = t*512 + g*128 + n holds the image that the
    # store writes to DRAM row ss*8192 + u*4096 + n*32 + h*4 + g, where
    # t = ss*16 + u*8 + h. Build the permutation via a reshape/transpose:
    # images viewed as [ss, u, n, h, g, p] -> column order (ss, u, h, g, n).
    n_ss = N_GROUPS // STORE_GROUPS
    n_u = STORE_GROUPS // INTER
    in_maps = []
    for c in range(N_CORES):
        xc = xf[c * PER_CORE : (c + 1) * PER_CORE]  # [32768, 128]
        xr = xc.reshape(n_ss, n_u, 128, INTER, G, PIX)  # [ss, u, n, h, g, p]
        # -> [p, ss, u, h, g, n]
        xt = xr.transpose(5, 0, 1, 3, 4, 2).reshape(PIX, PER_CORE)
        xt_bf = np.ascontiguousarray(xt).astype(ml_dtypes.bfloat16)
        in_maps.append({"xt": xt_bf, "w": w_bf})
    return in_maps


def _install_ntff_hook():
    """The agent image's antenv lacks axon_hooks; bass_utils needs it for
    trace=True. Register a ctypes-based hook module (same logic as
    trn_agent_boot.trn_boot._ntff_profile_via_ctypes)."""
    import types
    import ctypes
    import contextlib

    if "antenv.axon_hooks" in sys.modules:
        return True
    so_path = "/opt/axon/libaxon_pjrt.so"
    try:
        lib = ctypes.CDLL(so_path)
    except OSError:
        return False
    if not hasattr(lib, "axon_start_nrt_profile"):
        return False
    lib.axon_start_nrt_profile.argtypes = [
        ctypes.POINTER(ctypes.c_int64),
        ctypes.c_size_t,
    ]
    lib.axon_start_nrt_profile.restype = ctypes.c_int64
    lib.axon_stop_nrt_profile.argtypes = [ctypes.c_char_p]
    lib.axon_stop_nrt_profile.restype = ctypes.c_int64

    @contextlib.contextmanager
    def _hook(output_dir, device_ids):
        import jax

        jax.devices()
        if device_ids:
            ids = (ctypes.c_int64 * len(device_ids))(*device_ids)
            rc = lib.axon_start_nrt_profile(ids, len(device_ids))
        else:
            rc = lib.axon_start_nrt_profile(None, 0)
        if rc != 0:
            raise RuntimeError(f"axon_start_nrt_profile rc={rc}")
        try:
            yield
        finally:
            n = lib.axon_stop_nrt_profile(str(output_dir).encode())
            print(f"ntff profile: {n} file(s) written to {output_dir}")

    mod = types.ModuleType("antenv.axon_hooks")
    mod._hook = _hook
    mod.get_axon_ntff_profile_hook = lambda: _hook
    mod.set_axon_ntff_profile_hook = lambda h: None
    sys.modules["antenv.axon_hooks"] = mod
    import antenv

    antenv.axon_hooks = mod
    return True


def _run(x, kernel, trace=False):
    key = "nc"
    if key not in _COMPILED:
        _COMPILED[key] = _build_nc()
    nc = _COMPILED[key]
    in_maps = _prep_inputs(x, kernel)
    res = run_bass_kernel_spmd(
        nc, in_maps, core_ids=list(range(N_CORES)), trace=trace
    )
    outs = [np.asarray(res.results[c]["out"]) for c in range(N_CORES)]
    full = np.concatenate(outs, axis=0).astype(np.float32).reshape(B, S, OUT)
    return full, res


def kernel(x, kernel):
    out, _ = _run(x, kernel, trace=False)
    return out


def kernel_traced(x, kernel):
    """Same as kernel() but also returns BassKernelResults with trace info."""
    ok = _install_ntff_hook()
    if not ok:
        print("WARNING: could not install NTFF hook; running untraced")
    return _run(x, kernel, trace=ok)
